# revision 13
# baseline (speedup 1.0000x reference)
"""Trainium2 Bass kernel for nn_Net_43052752175597 (2-layer GraphSAGE, aggr=add).

reference:
    A_hat = (A != 0).T with unit diagonal          # [N, N], binary
    h1   = X @ W1;  agg1 = A_hat @ h1 + b1;  x1 = relu(l2norm(agg1))
    h2   = x1 @ W2; agg2 = A_hat @ h2 + b2;  out = l2norm(l2norm(agg2))

Sharding: row-shard A_hat (output nodes) across 8 cores; each core owns 1280
padded nodes (N 10000 -> 10240). Weights replicated.

Aggregation layout: h is the STATIONARY operand and the binary A streams as
the MOVING operand in fp8 (512 destination columns per matmul), producing
the aggregation feature-major ([feat, dst]). Layer-1 post-processing never
leaves feature-major: relu/bias fuse into one Activation pass (bias is
per-partition there), x1^T feeds the h2 matmul directly as the stationary
operand (no transposes), and per-node sum-of-squares comes from a
ones-vector matmul over bf16 squares. Only layer 2 transposes back to
node-major for the final output.

Precision: A is 0/1 so fp8 A tiles are exact. h is quantized per layer to
fp8 with a power-of-2 pre-scale (absorbed exactly by the downstream l2norm;
biases pre-scaled to match). The dense h matmuls run in plain bf16 (their
~1e-3 error is far below the fp8-h quantization error).
  AGG_MODE "e3e3": both layers fp8e3m4 h at 1 cyc/row     (rel err ~6.4e-3)
  AGG_MODE "e3dr": layer-2 h fp8e4m3 with DoubleRow pairs (rel err ~9.7e-3)

AllGathers are split (per-core node tiles 0:6 / 6:10) so aggregation over
the first chunk's sources overlaps the second transfer. The 13 MB/core A
slice is DMA'd in 8-k-tile batches on a separate SWDGE queue, k-ordered, so
layer-1 aggregation consumes it as it arrives.

PSUM discipline (hardware semantics): matmul start=True marks the whole
bank pending-zero lazily, so two accumulation groups must never interleave
while sharing a bank; single-shot matmuls (start+stop in one instruction)
may share a bank freely since reads are unaffected by pending-zero.
"""

import sys

sys.path.insert(0, "/opt/trn_rl_repo")

import numpy as np
import ml_dtypes

import concourse.bass as bass  # noqa: F401
import concourse.tile as tile
from concourse import bacc, mybir
from concourse import bass_utils

N = 10000
NP = 10240          # padded node count
F = 256             # input feature dim
H = 128             # hidden dim
N_CORES = 8
PER_CORE = NP // N_CORES        # 1280 nodes per core
M_TILES = PER_CORE // 128       # 10
K_TILES = NP // 128             # 80
MA = 6                          # m-tiles in gather chunk a
MB = M_TILES - MA               # 4
KA = N_CORES * MA               # 48 k-tiles in chunk a
KB = N_CORES * MB               # 32
CHUNKS = [(0, 512), (512, 1024), (1024, 1280)]   # dst column chunks
K_STREAM = K_TILES - 1   # korder[79] = core-7 m9: all padding, skip
ABATCH = 4          # k-tiles per A-load DMA

SC1 = 4.0           # h1 pre-scale (absorbed by l2norm; b1 scaled to match)
SC2 = 64.0          # h2 pre-scale

BF16 = ml_dtypes.bfloat16
E3M4 = ml_dtypes.float8_e3m4
E4M3 = ml_dtypes.float8_e4m3

AGG_MODE = "e3e3"

_CACHE = {}


def _h_dt(mode):
    return mybir.dt.float8e4 if mode == "dr" else mybir.dt.float8e3


def _build_nc(agg_mode=None, single_core=False, compile=True, repeats=1,
              fake_ag=False):
    """Build + compile the 8-core SPMD Bass kernel. Returns the Bacc object.

    single_core=True builds a 1-core variant with collectives replaced by
    equivalent-byte local DMAs — only for TimelineSim profiling.
    fake_ag=True keeps 8 cores but fakes the collectives the same way
    (WRONG results — collective-cost measurement only).
    repeats>1 re-runs the whole 2-layer body (benchmarking only).
    """
    agg_mode = agg_mode or AGG_MODE
    mode1, mode2 = agg_mode[:2], agg_mode[2:]
    assert mode1 == "e3" and mode2 in ("e3", "dr")
    fp32 = mybir.dt.float32
    bf16 = mybir.dt.bfloat16
    fp8a = mybir.dt.float8e4   # A tiles: binary, exact in any fp8
    dt1, dt2 = _h_dt(mode1), _h_dt(mode2)
    DR = mybir.MatmulPerfMode.DoubleRow
    Copy = mybir.ActivationFunctionType.Copy
    Relu = mybir.ActivationFunctionType.Relu
    Square = mybir.ActivationFunctionType.Square
    mult = mybir.AluOpType.mult

    nc = bacc.Bacc(
        "TRN2",
        target_bir_lowering=False,
        debug=False,
        enable_asserts=True,
        num_devices=1 if single_core else N_CORES,
    )

    a_pre = nc.dram_tensor("a_pre", [K_TILES, 128, PER_CORE], fp8a,
                           kind="ExternalInput").ap()
    xt = nc.dram_tensor("xt", [128, 2, PER_CORE], bf16,
                        kind="ExternalInput").ap()
    w1 = nc.dram_tensor("w1", [128, 2, H], bf16, kind="ExternalInput").ap()
    w2 = nc.dram_tensor("w2", [128, H], bf16, kind="ExternalInput").ap()
    b1c = nc.dram_tensor("b1c", [128, 1], fp32, kind="ExternalInput").ap()
    b2c = nc.dram_tensor("b2c", [128, 1], fp32, kind="ExternalInput").ap()
    b2h = nc.dram_tensor("b2h", [64, 2], fp32, kind="ExternalInput").ap()
    ones = nc.dram_tensor("ones", [128, 1], bf16, kind="ExternalInput").ap()
    ident = nc.dram_tensor("ident", [128, 128], fp32, kind="ExternalInput").ap()
    out = nc.dram_tensor("out", [PER_CORE, H], fp32, kind="ExternalOutput").ap()

    with tile.TileContext(nc) as tc:
        with tc.tile_pool(name="const", bufs=1) as cpool, \
             tc.tile_pool(name="acache", bufs=1) as apool, \
             tc.tile_pool(name="hfull", bufs=1) as hpool, \
             tc.tile_pool(name="work", bufs=1) as wpool, \
             tc.tile_pool(name="psum_agg", bufs=1, space="PSUM") as pagg, \
             tc.tile_pool(name="psum_mm", bufs=1, space="PSUM") as pmm, \
             tc.tile_pool(name="psum_ssq", bufs=1, space="PSUM") as pssq, \
             tc.tile_pool(name="psum_tr", bufs=1, space="PSUM") as ptr, \
             tc.tile_pool(name="dram", bufs=2, space="DRAM") as dpool:

            # ---- constants into SBUF ----
            t_xt = cpool.tile([128, 2, PER_CORE], bf16)
            t_w1 = cpool.tile([128, 2, H], bf16)
            t_w2 = cpool.tile([128, H], bf16)
            t_b1c = cpool.tile([128, 1], fp32)
            t_b2c = cpool.tile([128, 1], fp32)
            t_b2h = cpool.tile([64, 2], fp32)
            t_ones = cpool.tile([128, 1], bf16)
            t_id = cpool.tile([128, 128], fp32)
            nc.sync.dma_start(t_xt[:], xt[:])
            nc.sync.dma_start(t_w1[:], w1[:])
            nc.sync.dma_start(t_w2[:], w2[:])
            nc.sync.dma_start(t_b1c[:], b1c[:])
            nc.sync.dma_start(t_b2c[:], b2c[:])
            nc.sync.dma_start(t_b2h[:], b2h[:])
            nc.sync.dma_start(t_ones[:], ones[:])
            nc.sync.dma_start(t_id[:], ident[:])

            # whole per-core A slice, k-ordered (chunk-a k's first), batched
            # SWDGE loads so layer-1 agg consumes k-tiles as they arrive.
            t_a = apool.tile([128, K_TILES, PER_CORE], fp8a)
            for j0 in range(0, K_TILES, ABATCH):
                nc.gpsimd.dma_start(
                    t_a[:, j0:j0 + ABATCH, :],
                    a_pre[j0:j0 + ABATCH].rearrange("j p n -> p j n"))

            # gathered features (per layer dtype)
            t_h1a = hpool.tile([128, KA, H], dt1)
            t_h1b = hpool.tile([128, KB, H], dt1)
            t_h2a = hpool.tile([128, KA, H], dt2)
            t_h2b = hpool.tile([128, KB, H], dt2)

            # working tiles
            t_h1s = wpool.tile([128, M_TILES, H], dt1)    # own-shard h1
            t_h2s = wpool.tile([128, M_TILES, H], dt2)
            t_x1T = wpool.tile([128, PER_CORE], bf16)     # relu(agg1'), f-major
            t_sqT = wpool.tile([128, PER_CORE], bf16)     # squares, f-major
            t_row = wpool.tile([1, PER_CORE], fp32)       # ssq row staging
            t_ssqN = wpool.tile([128, M_TILES], fp32)     # dr-mode ssq accum
            t_nrm = wpool.tile([128, M_TILES], fp32)
            t_inv = wpool.tile([128, M_TILES], fp32)
            t_n2 = wpool.tile([128, M_TILES], fp32)
            t_inv2 = wpool.tile([128, M_TILES], fp32)
            t_scl = wpool.tile([128, M_TILES], fp32)
            t_sqs = wpool.tile([128, H], bf16)            # Square scratch (dr)
            t_aggN = wpool.tile([128, M_TILES, H], fp32)  # L2 node-major agg
            t_outf = wpool.tile([128, M_TILES, H], fp32)
            t_aggT = wpool.tile([128, PER_CORE], fp32)    # L2-e3 staging
            t_aggTh0 = wpool.tile([64, PER_CORE], fp32)   # L2-dr staging
            t_aggTh1 = wpool.tile([64, PER_CORE], fp32)
            t_aggTh = [t_aggTh0, t_aggTh1]

            # PSUM banks (see module docstring for sharing rules). The tile
            # framework serializes same-tile accumulation-group starts after
            # all prior reads of that tile, so latency-critical rotations
            # (h-matmuls, L2 transposes) alternate between TWO banks.
            t_ps_mm0 = pmm.tile([128, 512], fp32, name="t_ps_mm0")
            t_ps_mm1 = pmm.tile([128, 512], fp32, name="t_ps_mm1")
            _mm_banks = [t_ps_mm0, t_ps_mm1]
            t_ps_rowt = pssq.tile([1, 512], fp32, name="t_ps_rowt")
            t_ps_tr0 = ptr.tile([128, 512], fp32, name="t_ps_tr0")
            t_ps_tr1 = ptr.tile([128, 512], fp32, name="t_ps_tr1")
            _tr_banks = [t_ps_tr0, t_ps_tr1]

            def mm_slot(i):
                return _mm_banks[i % 2][:, 256 * ((i // 2) % 2):
                                        256 * ((i // 2) % 2) + H]

            def tr_slot(i):
                return _tr_banks[i % 2][:, 128 * ((i // 2) % 2):
                                        128 * ((i // 2) % 2) + 128]

            # transposed per-node ssq lives in tr-bank-0's spare columns
            # (256:266); all writers there are single-shot transposes.
            def ssq_cols(m0, m1):
                return t_ps_tr0[:, 256 + m0:256 + m1]

            def gather(t_hs, part, t_dst, h_dt, rep):
                """AllGather chunk ('a': m 0:6 | 'b': m 6:10) into t_dst."""
                m0, m1 = (0, MA) if part == "a" else (MA, M_TILES)
                rows = (m1 - m0) * 128
                kk = N_CORES * (m1 - m0)
                sh = dpool.tile([rows, H], h_dt, tag=f"sh_{part}",
                                name=f"sh_{part}_{rep}")
                g = dpool.tile([kk * 128, H], h_dt, tag=f"g_{part}",
                               addr_space="Shared", name=f"g_{part}_{rep}")
                nc.sync.dma_start(
                    sh[:].rearrange("(m p) f -> p m f", p=128),
                    t_hs[:, m0:m1, :])
                if single_core or fake_ag:
                    for c in range(N_CORES):
                        nc.sync.dma_start(
                            t_dst[:, c * (m1 - m0):(c + 1) * (m1 - m0), :],
                            sh[:].rearrange("(m p) f -> p m f", p=128))
                else:
                    nc.gpsimd.collective_compute(
                        "AllGather", mybir.AluOpType.bypass,
                        replica_groups=[list(range(N_CORES))],
                        ins=[sh.opt()], outs=[g.opt()],
                    )
                    nc.sync.dma_start(
                        t_dst[:], g[:].rearrange("(k p) f -> p k f", p=128))

            def h_of(t_pa, t_pb, j):
                return t_pa[:, j, :] if j < KA else t_pb[:, j - KA, :]

            def ssq_mm(ci, sq_src):
                """ones^T @ squares chunk -> ssq row psum; stage + transpose
                into the [128, M_TILES] node-major psum tile."""
                c0, c1 = CHUNKS[ci]
                nc.tensor.matmul(t_ps_rowt[:, 0:c1 - c0], t_ones[:],
                                 sq_src[:, c0:c1], start=True, stop=True)
                nc.vector.tensor_copy(t_row[:, c0:c1], t_ps_rowt[:, 0:c1 - c0])
                for m in range(c0 // 128, c1 // 128):
                    nc.tensor.matmul(
                        ssq_cols(m, m + 1),
                        t_row[:, m * 128:(m + 1) * 128], t_id[0:1, 0:1],
                        is_transpose=True, start=True, stop=True)

            def inv_chain(ssq_src, mr, with_l2l2=False):
                """t_inv[:, mr] = 1/max(sqrt(ssq), eps); optionally the
                double-l2norm composite scale. ssq_src: callable mr->AP."""
                nc.scalar.sqrt(t_nrm[:, mr], ssq_src(mr))
                nc.vector.tensor_scalar_max(t_nrm[:, mr], t_nrm[:, mr], 1e-12)
                nc.vector.reciprocal(t_inv[:, mr], t_nrm[:, mr])
                if with_l2l2:
                    # out = l2norm(l2norm(agg)): ||agg*inv|| = nrm*inv
                    nc.vector.tensor_tensor(t_n2[:, mr], t_nrm[:, mr],
                                            t_inv[:, mr], op=mult)
                    nc.vector.tensor_scalar_max(t_n2[:, mr], t_n2[:, mr],
                                                1e-12)
                    nc.vector.reciprocal(t_inv2[:, mr], t_n2[:, mr])
                    nc.vector.tensor_tensor(t_inv[:, mr], t_inv[:, mr],
                                            t_inv2[:, mr], op=mult)

            def out_m(m, src):
                nc.scalar.activation(t_outf[:, m, :], src, Copy,
                                     scale=t_inv[:, m:m + 1])
                nc.sync.dma_start(
                    out[:].rearrange("(mm p) f -> p mm f", p=128)[:, m, :],
                    t_outf[:, m, :])

            for _rep in range(repeats):
                # ====== Layer 1: h1 = X@W1 (bf16), quantize, chunked gather
                for m in range(M_TILES):
                    ps = mm_slot(m)
                    for k in range(2):
                        nc.tensor.matmul(ps, t_xt[:, k, m * 128:(m + 1) * 128],
                                         t_w1[:, k, :],
                                         start=(k == 0), stop=(k == 1))
                    nc.scalar.activation(t_h1s[:, m, :], ps, Copy, scale=SC1)
                    if m == MA - 1:
                        gather(t_h1s, "a", t_h1a, dt1, _rep)
                    elif m == M_TILES - 1:
                        gather(t_h1s, "b", t_h1b, dt1, _rep)

                # ====== Layer 1 aggregation: j-major (A arrival order),
                # 3 interleaved chunk groups on separate banks.
                af = [pagg.tile([128, 512], fp32, tag=f"af{ci}",
                                name=f"af{ci}_l1r{_rep}") for ci in range(3)]
                for j in range(K_STREAM):
                    hap = h_of(t_h1a, t_h1b, j)
                    for ci, (c0, c1) in enumerate(CHUNKS):
                        nc.tensor.matmul(af[ci][:, 0:c1 - c0], hap,
                                         t_a[:, j, c0:c1],
                                         start=(j == 0),
                                         stop=(j == K_STREAM - 1))

                # ====== Layer 1 post, all in feature-major:
                # x1^T = relu(agg'+b); squares -> ssq via ones-matmul;
                # h2 = x1@W2 with the l2norm scale folded into the quantize.
                for ci, (c0, c1) in enumerate(CHUNKS):
                    src = af[ci][:, 0:c1 - c0]
                    nc.scalar.activation(t_x1T[:, c0:c1], src, Relu,
                                         bias=t_b1c[:])
                    nc.scalar.activation(t_sqT[:, c0:c1], src, Square,
                                         bias=t_b1c[:])
                    ssq_mm(ci, t_sqT)
                inv_chain(lambda mr: ssq_cols(mr.start, mr.stop),
                          slice(0, M_TILES))
                nc.vector.tensor_scalar_mul(t_scl[:, :], t_inv[:, :], SC2)
                for m in range(M_TILES):
                    ps = mm_slot(m)
                    nc.tensor.matmul(ps, t_x1T[:, m * 128:(m + 1) * 128],
                                     t_w2[:], start=True, stop=True)
                    nc.scalar.activation(t_h2s[:, m, :], ps, Copy,
                                         scale=t_scl[:, m:m + 1])
                    if m == MA - 1:
                        gather(t_h2s, "a", t_h2a, dt2, _rep)
                    elif m == M_TILES - 1:
                        gather(t_h2s, "b", t_h2b, dt2, _rep)

                # ====== Layer 2 aggregation + post ======
                af2 = [pagg.tile([128, 512], fp32, tag=f"af{ci}",
                                 name=f"af{ci}_l2r{_rep}") for ci in range(3)]
                if mode2 == "e3":
                    # chunk-major passes so each chunk's post overlaps the
                    # next pass (bank reuse across passes is safe: the groups
                    # never interleave).
                    def l2_pass(ci):
                        c0, c1 = CHUNKS[ci]
                        for j in range(K_STREAM):
                            nc.tensor.matmul(
                                af2[ci][:, 0:c1 - c0],
                                h_of(t_h2a, t_h2b, j), t_a[:, j, c0:c1],
                                start=(j == 0), stop=(j == K_STREAM - 1))

                    def l2_post(ci):
                        c0, c1 = CHUNKS[ci]
                        src = af2[ci][:, 0:c1 - c0]
                        nc.vector.tensor_scalar_add(t_aggT[:, c0:c1], src,
                                                    t_b2c[:])
                        nc.scalar.activation(t_sqT[:, c0:c1], src, Square,
                                             bias=t_b2c[:])
                        ssq_mm(ci, t_sqT)
                        mr = slice(c0 // 128, c1 // 128)
                        inv_chain(lambda mr: ssq_cols(mr.start, mr.stop), mr,
                                  with_l2l2=True)
                        for m in range(c0 // 128, c1 // 128):
                            pst = tr_slot(m)
                            nc.tensor.transpose(
                                pst, t_aggT[:, m * 128:(m + 1) * 128],
                                t_id[:])
                            out_m(m, pst)

                    l2_pass(0)
                    l2_pass(1)
                    l2_post(0)
                    l2_pass(2)
                    l2_post(1)
                    l2_post(2)
                else:
                    # DoubleRow, half-major so the three chunk groups reuse
                    # the af banks across halves (half-1 starts only after
                    # half-0's staging copies read them).
                    n_sp = K_TILES // 2
                    for half in range(2):
                        for sp in range(n_sp):
                            j = 2 * sp
                            hsrc = (t_h2a[:, j:j + 2, :] if j < KA
                                    else t_h2b[:, j - KA:j - KA + 2, :])
                            lhsT = hsrc[:, :, 64 * half:64 * half + 64]
                            for ci, (c0, c1) in enumerate(CHUNKS):
                                nc.tensor.matmul(
                                    af2[ci][0:64, 0:c1 - c0], lhsT,
                                    t_a[:, j:j + 2, c0:c1], perf_mode=DR,
                                    start=(sp == 0), stop=(sp == n_sp - 1))
                        for ci, (c0, c1) in enumerate(CHUNKS):
                            nc.vector.tensor_scalar_add(
                                t_aggTh[half][:, c0:c1],
                                af2[ci][0:64, 0:c1 - c0],
                                t_b2h[:, half:half + 1])
                    for m in range(M_TILES):
                        pst = tr_slot(m)
                        for half in range(2):
                            nc.tensor.transpose(
                                pst[:, 64 * half:64 * half + 64],
                                t_aggTh[half][:, m * 128:(m + 1) * 128],
                                t_id[0:64, 0:64])
                        nc.vector.tensor_copy(t_aggN[:, m, :], pst)
                        nc.scalar.activation(t_sqs[:], pst, Square,
                                             accum_out=t_ssqN[:, m:m + 1])
                    inv_chain(lambda mr: t_ssqN[:, mr], slice(0, M_TILES),
                              with_l2l2=True)
                    for m in range(M_TILES):
                        out_m(m, t_aggN[:, m, :])

    if compile:
        nc.compile()
    return nc


def _prep_inputs(X, A, W1, b1, W2, b2, agg_mode=None):
    """Host-side sharding/layout prep. Returns in_maps for the 8 cores."""
    agg_mode = agg_mode or AGG_MODE
    f32 = np.float32

    # --- A_hat (source-major): Ab[j, i] = 1 iff edge j->i, unit diag ---
    Ab = np.zeros((NP, NP), dtype=E4M3)
    Ab[:N, :N] = (np.asarray(A) != 0)
    idx = np.arange(N)
    Ab[idx, idx] = 1.0

    # k-tile order: chunk-a tiles (per-core m 0:6) then chunk-b (m 6:10),
    # both core-major — matches the AllGather output layout.
    korder = ([c * M_TILES + m for c in range(N_CORES) for m in range(MA)]
              + [c * M_TILES + m for c in range(N_CORES)
                 for m in range(MA, M_TILES)])

    # --- X^T (bf16), padded ---
    Xp = np.zeros((NP, F), dtype=f32)
    Xp[:N] = np.asarray(X, dtype=f32)
    XT = np.ascontiguousarray(Xp.T).astype(BF16)      # [256, NP]

    w1_host = np.ascontiguousarray(
        np.asarray(W1, dtype=f32).reshape(2, 128, H)
        .transpose(1, 0, 2)).astype(BF16)              # [128, 2, H]
    w2_host = np.asarray(W2, dtype=f32).astype(BF16)   # [128, H]

    b1s = SC1 * np.asarray(b1, dtype=f32)
    b2s = SC2 * np.asarray(b2, dtype=f32)
    b1c = np.ascontiguousarray(b1s.reshape(128, 1))
    b2c = np.ascontiguousarray(b2s.reshape(128, 1))
    b2h = np.ascontiguousarray(b2s.reshape(2, 64).T)   # [64, 2]
    ones_host = np.ones((128, 1), dtype=BF16)
    ident = np.eye(128, dtype=f32)

    in_maps = []
    for c in range(N_CORES):
        cols = slice(c * PER_CORE, (c + 1) * PER_CORE)
        S = Ab[:, cols].reshape(K_TILES, 128, PER_CORE)
        a_pre_c = np.ascontiguousarray(S[korder])
        xt_c = np.ascontiguousarray(
            XT[:, cols].reshape(2, 128, PER_CORE).transpose(1, 0, 2))
        in_maps.append({
            "a_pre": a_pre_c,
            "xt": xt_c,
            "w1": w1_host,
            "w2": w2_host,
            "b1c": b1c,
            "b2c": b2c,
            "b2h": b2h,
            "ones": ones_host,
            "ident": ident,
        })
    return in_maps


def _get_nc(agg_mode=None):
    agg_mode = agg_mode or AGG_MODE
    key = f"nc_{agg_mode}"
    if key not in _CACHE:
        _CACHE[key] = _build_nc(agg_mode)
    return _CACHE[key]


def kernel(X, A, W1, b1, W2, b2, _trace=False, _trace_kwargs=None):
    nc = _get_nc()
    in_maps = _prep_inputs(X, A, W1, b1, W2, b2, AGG_MODE)
    kw = {}
    if _trace:
        kw.update(trace=True, **(_trace_kwargs or {}))
    res = bass_utils.run_bass_kernel_spmd(
        nc, in_maps, core_ids=list(range(N_CORES)), **kw)
    _CACHE["last_result"] = res
    out = np.concatenate([res.results[c]["out"] for c in range(N_CORES)],
                         axis=0)[:N]
    return np.ascontiguousarray(out.astype(np.float32))


# revision 14
# speedup vs baseline: 1.5960x; 1.5960x over previous
"""Trainium2 Bass kernel for nn_Net_43052752175597 (2-layer GraphSAGE, aggr=add).

reference:
    A_hat = (A != 0).T with unit diagonal          # [N, N], binary
    h1   = X @ W1;  agg1 = A_hat @ h1 + b1;  x1 = relu(l2norm(agg1))
    h2   = x1 @ W2; agg2 = A_hat @ h2 + b2;  out = l2norm(l2norm(agg2))

Sharding: row-shard A_hat (output nodes) across 8 cores; each core owns 1280
padded nodes (N 10000 -> 10240). Weights replicated.

Aggregation layout: h is the STATIONARY operand and the binary A streams as
the MOVING operand in fp8 (512 destination columns per matmul), producing
the aggregation feature-major ([feat, dst]). Layer-1 post-processing never
leaves feature-major: relu/bias fuse into one Activation pass (bias is
per-partition there), x1^T feeds the h2 matmul directly as the stationary
operand (no transposes), and per-node sum-of-squares comes from a
ones-vector matmul over bf16 squares. Only layer 2 transposes back to
node-major for the final output.

Precision: A is 0/1 so fp8 A tiles are exact. h is quantized per layer to
fp8 with a power-of-2 pre-scale (absorbed exactly by the downstream l2norm;
biases pre-scaled to match). The dense h matmuls run in plain bf16 (their
~1e-3 error is far below the fp8-h quantization error).
  AGG_MODE "e3e3": both layers fp8e3m4 h at 1 cyc/row     (rel err ~6.4e-3)
  AGG_MODE "e3dr": layer-2 h fp8e4m3 with DoubleRow pairs (rel err ~9.7e-3)

AllGathers are split (per-core node tiles 0:6 / 6:10) so aggregation over
the first chunk's sources overlaps the second transfer. The 13 MB/core A
slice is DMA'd in 8-k-tile batches on a separate SWDGE queue, k-ordered, so
layer-1 aggregation consumes it as it arrives.

PSUM discipline (hardware semantics): matmul start=True marks the whole
bank pending-zero lazily, so two accumulation groups must never interleave
while sharing a bank; single-shot matmuls (start+stop in one instruction)
may share a bank freely since reads are unaffected by pending-zero.
"""

import sys

sys.path.insert(0, "/opt/trn_rl_repo")

import numpy as np
import ml_dtypes

import concourse.bass as bass  # noqa: F401
import concourse.tile as tile
from concourse import bacc, mybir
from concourse import bass_utils

N = 10000
NP = 10240          # padded node count
F = 256             # input feature dim
H = 128             # hidden dim
N_CORES = 8
PER_CORE = NP // N_CORES        # 1280 nodes per core
M_TILES = PER_CORE // 128       # 10
K_TILES = NP // 128             # 80
MA = 6                          # m-tiles in gather chunk a
MB = M_TILES - MA               # 4
KA = N_CORES * MA               # 48 k-tiles in chunk a
KB = N_CORES * MB               # 32
CHUNKS = [(0, 512), (512, 1024), (1024, 1280)]   # dst column chunks
K_STREAM = K_TILES - 1   # korder[79] = core-7 m9: all padding, skip
ABATCH = 4          # k-tiles per A-load DMA

SC1 = 4.0           # h1 pre-scale (absorbed by l2norm; b1 scaled to match)
SC2 = 64.0          # h2 pre-scale

BF16 = ml_dtypes.bfloat16
E3M4 = ml_dtypes.float8_e3m4
E4M3 = ml_dtypes.float8_e4m3

AGG_MODE = "e3e3"

_CACHE = {}


def _h_dt(mode):
    return mybir.dt.float8e4 if mode == "dr" else mybir.dt.float8e3


def _build_nc(agg_mode=None, single_core=False, compile=True, repeats=1,
              fake_ag=False):
    """Build + compile the 8-core SPMD Bass kernel. Returns the Bacc object.

    single_core=True builds a 1-core variant with collectives replaced by
    equivalent-byte local DMAs — only for TimelineSim profiling.
    fake_ag=True keeps 8 cores but fakes the collectives the same way
    (WRONG results — collective-cost measurement only).
    repeats>1 re-runs the whole 2-layer body (benchmarking only).
    """
    agg_mode = agg_mode or AGG_MODE
    mode1, mode2 = agg_mode[:2], agg_mode[2:]
    assert mode1 == "e3" and mode2 in ("e3", "dr")
    fp32 = mybir.dt.float32
    bf16 = mybir.dt.bfloat16
    fp8a = mybir.dt.float8e4   # A tiles: binary, exact in any fp8
    dt1, dt2 = _h_dt(mode1), _h_dt(mode2)
    DR = mybir.MatmulPerfMode.DoubleRow
    Copy = mybir.ActivationFunctionType.Copy
    Relu = mybir.ActivationFunctionType.Relu
    Square = mybir.ActivationFunctionType.Square
    mult = mybir.AluOpType.mult

    nc = bacc.Bacc(
        "TRN2",
        target_bir_lowering=False,
        debug=False,
        enable_asserts=True,
        num_devices=1 if single_core else N_CORES,
    )

    a_pre = nc.dram_tensor("a_pre", [K_TILES, 128, PER_CORE], fp8a,
                           kind="ExternalInput").ap()
    xt = nc.dram_tensor("xt", [128, 2, PER_CORE], bf16,
                        kind="ExternalInput").ap()
    w1 = nc.dram_tensor("w1", [128, 2, H], bf16, kind="ExternalInput").ap()
    w2 = nc.dram_tensor("w2", [128, H], bf16, kind="ExternalInput").ap()
    b1c = nc.dram_tensor("b1c", [128, 1], fp32, kind="ExternalInput").ap()
    b2c = nc.dram_tensor("b2c", [128, 1], fp32, kind="ExternalInput").ap()
    b2h = nc.dram_tensor("b2h", [64, 2], fp32, kind="ExternalInput").ap()
    ones = nc.dram_tensor("ones", [128, 1], bf16, kind="ExternalInput").ap()
    ident = nc.dram_tensor("ident", [128, 128], fp32, kind="ExternalInput").ap()
    out = nc.dram_tensor("out", [PER_CORE, H], fp32, kind="ExternalOutput").ap()

    with tile.TileContext(nc) as tc:
        with tc.tile_pool(name="const", bufs=1) as cpool, \
             tc.tile_pool(name="acache", bufs=1) as apool, \
             tc.tile_pool(name="hfull", bufs=1) as hpool, \
             tc.tile_pool(name="work", bufs=1) as wpool, \
             tc.tile_pool(name="psum_agg", bufs=1, space="PSUM") as pagg, \
             tc.tile_pool(name="psum_mm", bufs=1, space="PSUM") as pmm, \
             tc.tile_pool(name="psum_ssq", bufs=1, space="PSUM") as pssq, \
             tc.tile_pool(name="psum_tr", bufs=1, space="PSUM") as ptr, \
             tc.tile_pool(name="dram", bufs=2, space="DRAM") as dpool:

            # ---- constants into SBUF ----
            t_xt = cpool.tile([128, 2, PER_CORE], bf16)
            t_w1 = cpool.tile([128, 2, H], bf16)
            t_w2 = cpool.tile([128, H], bf16)
            t_b1c = cpool.tile([128, 1], fp32)
            t_b2c = cpool.tile([128, 1], fp32)
            t_b2h = cpool.tile([64, 2], fp32)
            t_ones = cpool.tile([128, 1], bf16)
            t_id = cpool.tile([128, 128], fp32)
            nc.sync.dma_start(t_xt[:, :, 0:MA * 128], xt[:, :, 0:MA * 128])
            nc.sync.dma_start(t_xt[:, :, MA * 128:], xt[:, :, MA * 128:])
            nc.sync.dma_start(t_w1[:], w1[:])
            nc.sync.dma_start(t_w2[:], w2[:])
            nc.sync.dma_start(t_b1c[:], b1c[:])
            nc.sync.dma_start(t_b2c[:], b2c[:])
            nc.sync.dma_start(t_b2h[:], b2h[:])
            nc.sync.dma_start(t_ones[:], ones[:])
            nc.sync.dma_start(t_id[:], ident[:])

            # whole per-core A slice, k-ordered (chunk-a k's first), batched
            # SWDGE loads so layer-1 agg consumes k-tiles as they arrive.
            t_a = apool.tile([128, K_TILES, PER_CORE], fp8a)
            for j0 in range(0, K_TILES, ABATCH):
                nc.gpsimd.dma_start(
                    t_a[:, j0:j0 + ABATCH, :],
                    a_pre[j0:j0 + ABATCH].rearrange("j p n -> p j n"))

            # gathered features (per layer dtype)
            t_h1a = hpool.tile([128, KA, H], dt1)
            t_h1b = hpool.tile([128, KB, H], dt1)
            t_h2a = hpool.tile([128, KA, H], dt2)
            t_h2b = hpool.tile([128, KB, H], dt2)

            # working tiles
            t_h1s = wpool.tile([128, M_TILES, H], dt1)    # own-shard h1
            t_h2s = wpool.tile([128, M_TILES, H], dt2)
            t_x1T = wpool.tile([128, PER_CORE], bf16)     # relu(agg1'), f-major
            t_sqT = wpool.tile([128, PER_CORE], bf16)     # squares, f-major
            t_row = wpool.tile([1, PER_CORE], fp32)       # ssq row staging
            t_ssqN = wpool.tile([128, M_TILES], fp32)     # dr-mode ssq accum
            t_nrm = wpool.tile([128, M_TILES], fp32)
            t_inv = wpool.tile([128, M_TILES], fp32)
            t_n2 = wpool.tile([128, M_TILES], fp32)
            t_inv2 = wpool.tile([128, M_TILES], fp32)
            t_scl = wpool.tile([128, M_TILES], fp32)
            t_sqs = wpool.tile([128, H], bf16)            # Square scratch (dr)
            t_aggN = wpool.tile([128, M_TILES, H], fp32)  # L2 node-major agg
            t_outf = wpool.tile([128, M_TILES, H], fp32)
            t_aggT = wpool.tile([128, PER_CORE], fp32)    # L2-e3 staging
            t_aggTh0 = wpool.tile([64, PER_CORE], fp32)   # L2-dr staging
            t_aggTh1 = wpool.tile([64, PER_CORE], fp32)
            t_aggTh = [t_aggTh0, t_aggTh1]

            # PSUM banks (see module docstring for sharing rules). The tile
            # framework serializes same-tile accumulation-group starts after
            # all prior reads of that tile, so latency-critical rotations
            # (h-matmuls, L2 transposes) alternate between TWO banks.
            t_ps_mm0 = pmm.tile([128, 512], fp32, name="t_ps_mm0")
            t_ps_mm1 = pmm.tile([128, 512], fp32, name="t_ps_mm1")
            _mm_banks = [t_ps_mm0, t_ps_mm1]
            t_ps_rowt = pssq.tile([1, 512], fp32, name="t_ps_rowt")
            t_ps_tr0 = ptr.tile([128, 512], fp32, name="t_ps_tr0")
            t_ps_tr1 = ptr.tile([128, 512], fp32, name="t_ps_tr1")
            _tr_banks = [t_ps_tr0, t_ps_tr1]

            def mm_slot(i):
                return _mm_banks[i % 2][:, 256 * ((i // 2) % 2):
                                        256 * ((i // 2) % 2) + H]

            def tr_slot(i):
                return _tr_banks[i % 2][:, 128 * ((i // 2) % 2):
                                        128 * ((i // 2) % 2) + 128]

            # transposed per-node ssq lives in tr-bank-0's spare columns
            # (256:266); all writers there are single-shot transposes.
            def ssq_cols(m0, m1):
                return t_ps_tr0[:, 256 + m0:256 + m1]

            def gather(t_hs, part, t_dst, h_dt, rep):
                """AllGather chunk ('a': m 0:6 | 'b': m 6:10) into t_dst."""
                m0, m1 = (0, MA) if part == "a" else (MA, M_TILES)
                rows = (m1 - m0) * 128
                kk = N_CORES * (m1 - m0)
                sh = dpool.tile([rows, H], h_dt, tag=f"sh_{part}",
                                name=f"sh_{part}_{rep}")
                g = dpool.tile([kk * 128, H], h_dt, tag=f"g_{part}",
                               addr_space="Shared", name=f"g_{part}_{rep}")
                nc.sync.dma_start(
                    sh[:].rearrange("(m p) f -> p m f", p=128),
                    t_hs[:, m0:m1, :])
                if single_core or fake_ag:
                    for c in range(N_CORES):
                        nc.sync.dma_start(
                            t_dst[:, c * (m1 - m0):(c + 1) * (m1 - m0), :],
                            sh[:].rearrange("(m p) f -> p m f", p=128))
                else:
                    nc.gpsimd.collective_compute(
                        "AllGather", mybir.AluOpType.bypass,
                        replica_groups=[list(range(N_CORES))],
                        ins=[sh.opt()], outs=[g.opt()],
                    )
                    nc.sync.dma_start(
                        t_dst[:], g[:].rearrange("(k p) f -> p k f", p=128))

            def h_of(t_pa, t_pb, j):
                return t_pa[:, j, :] if j < KA else t_pb[:, j - KA, :]

            def ssq_mm(ci, sq_src):
                """ones^T @ squares chunk -> ssq row psum; stage + transpose
                into the [128, M_TILES] node-major psum tile."""
                c0, c1 = CHUNKS[ci]
                nc.tensor.matmul(t_ps_rowt[:, 0:c1 - c0], t_ones[:],
                                 sq_src[:, c0:c1], start=True, stop=True)
                nc.vector.tensor_copy(t_row[:, c0:c1], t_ps_rowt[:, 0:c1 - c0])
                for m in range(c0 // 128, c1 // 128):
                    nc.tensor.matmul(
                        ssq_cols(m, m + 1),
                        t_row[:, m * 128:(m + 1) * 128], t_id[0:1, 0:1],
                        is_transpose=True, start=True, stop=True)

            def inv_chain(ssq_src, mr, with_l2l2=False):
                """t_inv[:, mr] = 1/max(sqrt(ssq), eps); optionally the
                double-l2norm composite scale. ssq_src: callable mr->AP."""
                nc.scalar.sqrt(t_nrm[:, mr], ssq_src(mr))
                nc.vector.tensor_scalar_max(t_nrm[:, mr], t_nrm[:, mr], 1e-12)
                nc.vector.reciprocal(t_inv[:, mr], t_nrm[:, mr])
                if with_l2l2:
                    # out = l2norm(l2norm(agg)): ||agg*inv|| = nrm*inv
                    nc.vector.tensor_tensor(t_n2[:, mr], t_nrm[:, mr],
                                            t_inv[:, mr], op=mult)
                    nc.vector.tensor_scalar_max(t_n2[:, mr], t_n2[:, mr],
                                                1e-12)
                    nc.vector.reciprocal(t_inv2[:, mr], t_n2[:, mr])
                    nc.vector.tensor_tensor(t_inv[:, mr], t_inv[:, mr],
                                            t_inv2[:, mr], op=mult)

            def out_m(m, src, dma=True):
                if m % 2 == 0:
                    nc.scalar.activation(t_outf[:, m, :], src, Copy,
                                         scale=t_inv[:, m:m + 1])
                else:
                    nc.vector.tensor_scalar_mul(t_outf[:, m, :], src,
                                                t_inv[:, m:m + 1])
                if dma:
                    nc.sync.dma_start(
                        out[:].rearrange("(mm p) f -> p mm f", p=128)[:, m, :],
                        t_outf[:, m, :])

            for _rep in range(repeats):
                # ====== Layer 1: h1 = X@W1 (bf16), quantize, chunked gather
                for m in range(M_TILES):
                    ps = mm_slot(m)
                    for k in range(2):
                        nc.tensor.matmul(ps, t_xt[:, k, m * 128:(m + 1) * 128],
                                         t_w1[:, k, :],
                                         start=(k == 0), stop=(k == 1))
                    nc.scalar.activation(t_h1s[:, m, :], ps, Copy, scale=SC1)
                    if m == MA - 1:
                        gather(t_h1s, "a", t_h1a, dt1, _rep)
                    elif m == M_TILES - 1:
                        gather(t_h1s, "b", t_h1b, dt1, _rep)

                # ====== Layer 1 aggregation: j-major (A arrival order),
                # 3 interleaved chunk groups on separate banks.
                af = [pagg.tile([128, 512], fp32, tag=f"af{ci}",
                                name=f"af{ci}_l1r{_rep}") for ci in range(3)]
                for j in range(K_STREAM):
                    hap = h_of(t_h1a, t_h1b, j)
                    for ci, (c0, c1) in enumerate(CHUNKS):
                        nc.tensor.matmul(af[ci][:, 0:c1 - c0], hap,
                                         t_a[:, j, c0:c1],
                                         start=(j == 0),
                                         stop=(j == K_STREAM - 1))

                # ====== Layer 1 post, all in feature-major:
                # x1^T = relu(agg'+b); squares -> ssq via ones-matmul;
                # h2 = x1@W2 with the l2norm scale folded into the quantize.
                for ci, (c0, c1) in enumerate(CHUNKS):
                    src = af[ci][:, 0:c1 - c0]
                    nc.vector.tensor_scalar(t_x1T[:, c0:c1], src, t_b1c[:],
                                            0.0, op0=mybir.AluOpType.add,
                                            op1=mybir.AluOpType.max)
                    nc.scalar.activation(t_sqT[:, c0:c1], src, Square,
                                         bias=t_b1c[:])
                    ssq_mm(ci, t_sqT)
                inv_chain(lambda mr: ssq_cols(mr.start, mr.stop),
                          slice(0, M_TILES))
                nc.vector.tensor_scalar_mul(t_scl[:, :], t_inv[:, :], SC2)
                for m in range(M_TILES):
                    ps = mm_slot(m)
                    nc.tensor.matmul(ps, t_x1T[:, m * 128:(m + 1) * 128],
                                     t_w2[:], start=True, stop=True)
                    if m % 2 == 0:
                        nc.scalar.activation(t_h2s[:, m, :], ps, Copy,
                                             scale=t_scl[:, m:m + 1])
                    else:
                        nc.vector.tensor_scalar_mul(t_h2s[:, m, :], ps,
                                                    t_scl[:, m:m + 1])
                    if m == MA - 1:
                        gather(t_h2s, "a", t_h2a, dt2, _rep)
                    elif m == M_TILES - 1:
                        gather(t_h2s, "b", t_h2b, dt2, _rep)

                # ====== Layer 2 aggregation + post ======
                af2 = [pagg.tile([128, 512], fp32, tag=f"af{ci}",
                                 name=f"af{ci}_l2r{_rep}") for ci in range(3)]
                if mode2 == "e3":
                    # chunk-major passes so each chunk's post overlaps the
                    # next pass (bank reuse across passes is safe: the groups
                    # never interleave).
                    def l2_pass(ci):
                        c0, c1 = CHUNKS[ci]
                        for j in range(K_STREAM):
                            nc.tensor.matmul(
                                af2[ci][:, 0:c1 - c0],
                                h_of(t_h2a, t_h2b, j), t_a[:, j, c0:c1],
                                start=(j == 0), stop=(j == K_STREAM - 1))

                    def l2_post(ci):
                        c0, c1 = CHUNKS[ci]
                        src = af2[ci][:, 0:c1 - c0]
                        nc.vector.tensor_scalar_add(t_aggT[:, c0:c1], src,
                                                    t_b2c[:])
                        nc.scalar.activation(t_sqT[:, c0:c1], src, Square,
                                             bias=t_b2c[:])
                        ssq_mm(ci, t_sqT)
                        mr = slice(c0 // 128, c1 // 128)
                        inv_chain(lambda mr: ssq_cols(mr.start, mr.stop), mr,
                                  with_l2l2=True)
                        for m in range(c0 // 128, c1 // 128):
                            pst = tr_slot(m)
                            nc.tensor.transpose(
                                pst, t_aggT[:, m * 128:(m + 1) * 128],
                                t_id[:])
                            out_m(m, pst, dma=False)
                        nc.sync.dma_start(
                            out[:].rearrange(
                                "(mm p) f -> p mm f",
                                p=128)[:, c0 // 128:c1 // 128, :],
                            t_outf[:, c0 // 128:c1 // 128, :])

                    l2_pass(0)
                    l2_pass(1)
                    l2_post(0)
                    l2_pass(2)
                    l2_post(1)
                    l2_post(2)
                else:
                    # DoubleRow, half-major so the three chunk groups reuse
                    # the af banks across halves (half-1 starts only after
                    # half-0's staging copies read them).
                    n_sp = K_TILES // 2
                    for half in range(2):
                        for sp in range(n_sp):
                            j = 2 * sp
                            hsrc = (t_h2a[:, j:j + 2, :] if j < KA
                                    else t_h2b[:, j - KA:j - KA + 2, :])
                            lhsT = hsrc[:, :, 64 * half:64 * half + 64]
                            for ci, (c0, c1) in enumerate(CHUNKS):
                                nc.tensor.matmul(
                                    af2[ci][0:64, 0:c1 - c0], lhsT,
                                    t_a[:, j:j + 2, c0:c1], perf_mode=DR,
                                    start=(sp == 0), stop=(sp == n_sp - 1))
                        for ci, (c0, c1) in enumerate(CHUNKS):
                            nc.vector.tensor_scalar_add(
                                t_aggTh[half][:, c0:c1],
                                af2[ci][0:64, 0:c1 - c0],
                                t_b2h[:, half:half + 1])
                    for m in range(M_TILES):
                        pst = tr_slot(m)
                        for half in range(2):
                            nc.tensor.transpose(
                                pst[:, 64 * half:64 * half + 64],
                                t_aggTh[half][:, m * 128:(m + 1) * 128],
                                t_id[0:64, 0:64])
                        nc.vector.tensor_copy(t_aggN[:, m, :], pst)
                        nc.scalar.activation(t_sqs[:], pst, Square,
                                             accum_out=t_ssqN[:, m:m + 1])
                    inv_chain(lambda mr: t_ssqN[:, mr], slice(0, M_TILES),
                              with_l2l2=True)
                    for m in range(M_TILES):
                        out_m(m, t_aggN[:, m, :])

    if compile:
        nc.compile()
    return nc


def _prep_inputs(X, A, W1, b1, W2, b2, agg_mode=None):
    """Host-side sharding/layout prep. Returns in_maps for the 8 cores."""
    agg_mode = agg_mode or AGG_MODE
    f32 = np.float32

    # --- A_hat (source-major): Ab[j, i] = 1 iff edge j->i, unit diag ---
    Ab = np.zeros((NP, NP), dtype=E4M3)
    Ab[:N, :N] = (np.asarray(A) != 0)
    idx = np.arange(N)
    Ab[idx, idx] = 1.0

    # k-tile order: chunk-a tiles (per-core m 0:6) then chunk-b (m 6:10),
    # both core-major — matches the AllGather output layout.
    korder = ([c * M_TILES + m for c in range(N_CORES) for m in range(MA)]
              + [c * M_TILES + m for c in range(N_CORES)
                 for m in range(MA, M_TILES)])

    # --- X^T (bf16), padded ---
    Xp = np.zeros((NP, F), dtype=f32)
    Xp[:N] = np.asarray(X, dtype=f32)
    XT = np.ascontiguousarray(Xp.T).astype(BF16)      # [256, NP]

    w1_host = np.ascontiguousarray(
        np.asarray(W1, dtype=f32).reshape(2, 128, H)
        .transpose(1, 0, 2)).astype(BF16)              # [128, 2, H]
    w2_host = np.asarray(W2, dtype=f32).astype(BF16)   # [128, H]

    b1s = SC1 * np.asarray(b1, dtype=f32)
    b2s = SC2 * np.asarray(b2, dtype=f32)
    b1c = np.ascontiguousarray(b1s.reshape(128, 1))
    b2c = np.ascontiguousarray(b2s.reshape(128, 1))
    b2h = np.ascontiguousarray(b2s.reshape(2, 64).T)   # [64, 2]
    ones_host = np.ones((128, 1), dtype=BF16)
    ident = np.eye(128, dtype=f32)

    in_maps = []
    for c in range(N_CORES):
        cols = slice(c * PER_CORE, (c + 1) * PER_CORE)
        S = Ab[:, cols].reshape(K_TILES, 128, PER_CORE)
        a_pre_c = np.ascontiguousarray(S[korder])
        xt_c = np.ascontiguousarray(
            XT[:, cols].reshape(2, 128, PER_CORE).transpose(1, 0, 2))
        in_maps.append({
            "a_pre": a_pre_c,
            "xt": xt_c,
            "w1": w1_host,
            "w2": w2_host,
            "b1c": b1c,
            "b2c": b2c,
            "b2h": b2h,
            "ones": ones_host,
            "ident": ident,
        })
    return in_maps


def _get_nc(agg_mode=None):
    agg_mode = agg_mode or AGG_MODE
    key = f"nc_{agg_mode}"
    if key not in _CACHE:
        _CACHE[key] = _build_nc(agg_mode)
    return _CACHE[key]


def kernel(X, A, W1, b1, W2, b2, _trace=False, _trace_kwargs=None):
    nc = _get_nc()
    in_maps = _prep_inputs(X, A, W1, b1, W2, b2, AGG_MODE)
    kw = {}
    if _trace:
        kw.update(trace=True, **(_trace_kwargs or {}))
    res = bass_utils.run_bass_kernel_spmd(
        nc, in_maps, core_ids=list(range(N_CORES)), **kw)
    _CACHE["last_result"] = res
    out = np.concatenate([res.results[c]["out"] for c in range(N_CORES)],
                         axis=0)[:N]
    return np.ascontiguousarray(out.astype(np.float32))
